# revision 1
# baseline (speedup 1.0000x reference)
"""Trainium2 Bass kernel for the binarized-conv bottleneck block.

Math: out = prelu(prelu(bn3(bconv3(s3))) + x), where
  s1 = binarize(x); c1 = bconv1(s1) (1x1, 128->32)
  s2 = binarize(bn1(c1))  (prelu dropped: it preserves sign)
  c2 = bconv2(s2) (3x3 pad 1, 32->32); s3 = binarize(bn2(c2))
  c3 = bconv3(s3) (1x1, 32->128)

Tricks:
- Binarized values carried as g in {0,1} (fp8), s = 2g-1; conv weights are
  2*sign(w), so PSUM gets c' = c_ref + rowsum(sign(w)); rowsum folds into
  per-channel thresholds / the BN3 bias. 3x3 zero-padding is g = 0.5.
  All values are exact in fp8/f32, so the kernel matches the reference to
  f32 rounding.
- The image is split into 4 row-bands of 32, one per partition group
  (32j..32j+32). Bands 0,2 sit on partition groups 0,1 and are processed
  top-down; bands 1,3 sit on groups 2,3 and run bottom-up (boustrophedon),
  which makes every band-boundary halo row available exactly when needed.
  The stage-1 threshold then writes the padded conv2 input slab for each
  band directly (lane-aligned, no partition-crossing), and the 3x3 conv is
  9 shifted K=32 matmuls on 4 concurrent diagonal PE tiles.

Sharding: data-parallel over batch, one image per NeuronCore (8 cores).
"""
import numpy as np
import ml_dtypes

import concourse.bass as bass
import concourse.mybir as mybir
from concourse import bacc
from concourse.tile import TileContext
from concourse.bass_utils import run_bass_kernel_spmd

F32 = mybir.dt.float32
FP8 = mybir.dt.float8e4
AF = mybir.ActivationFunctionType
OP = mybir.AluOpType

B, C, CI, H, W = 8, 128, 32, 128, 256
HW = H * W                    # 32768
BH = 32                       # band height (rows per band)
WP = W + 2                    # 258
SLAB = BH + 2                 # 34 rows: halo + 32 interior + halo
MACRO = 2048                  # pixels per macro = 2 rows x 4 bands
NMAC = 16
EPS = 1e-5

# group g (partitions 32g..) -> band; groups 0,1 top-down; 2,3 bottom-up
BAND_OF_GROUP = (0, 2, 1, 3)

_CACHE = {}

CFG = dict(g1="vector", radd="vector")


def _build(a3: float, a_out: float, repeat: int = 1, variant: str = "full"):
    nc = bacc.Bacc("TRN2", debug=False)

    x_d = nc.dram_tensor("x", [C, HW], F32, kind="ExternalInput")
    w1_d = nc.dram_tensor("w1s", [C, CI], FP8, kind="ExternalInput")
    w2_d = nc.dram_tensor("w2s", [C, 9 * CI], FP8, kind="ExternalInput")
    w3_d = nc.dram_tensor("w3s", [C, C], FP8, kind="ExternalInput")
    vec_d = nc.dram_tensor("vecs", [C, 4], F32, kind="ExternalInput")
    idn_d = nc.dram_tensor("idn", [C, C], F32, kind="ExternalInput")
    out_d = nc.dram_tensor("out", [C, HW], F32, kind="ExternalOutput")

    with TileContext(nc) as tc:
        with (
            tc.tile_pool(name="const", bufs=1) as cpool,
            tc.tile_pool(name="res", bufs=1) as rpool,
            tc.tile_pool(name="work", bufs=int(CFG.get("wb", 3))) as wpool,
            tc.tile_pool(name="eplg", bufs=int(CFG.get("eb", 3))) as epool,
            tc.tile_pool(name="ps1", bufs=2, space="PSUM") as ps1,
            tc.tile_pool(name="ps2", bufs=2, space="PSUM") as ps2,
            tc.tile_pool(name="ps3", bufs=2, space="PSUM") as ps3,
        ):
            # ---- constants ----
            w1s = cpool.tile([C, CI], FP8)
            nc.sync.dma_start(out=w1s, in_=w1_d[:, :])
            w2s = cpool.tile([C, 9 * CI], FP8)
            nc.sync.dma_start(out=w2s, in_=w2_d[:, :])
            w3s = cpool.tile([C, C], FP8)
            nc.sync.dma_start(out=w3s, in_=w3_d[:, :])
            vecs = cpool.tile([C, 4], F32)
            nc.sync.dma_start(out=vecs, in_=vec_d[:, :])
            idn = cpool.tile([C, C], F32)
            nc.sync.dma_start(out=idn, in_=idn_d[:, :])
            t1v = vecs[:, 0:1]
            t2v = vecs[:, 1:2]
            sc3v = vecs[:, 2:3]
            b3v = vecs[:, 3:4]

            # ---- residents ----
            x_sb = rpool.tile([C, HW], F32)
            xv4 = x_sb.rearrange("p (b r) -> p b r", b=4)      # band view
            ov4 = out_d[:, :].rearrange("p (b r) -> p b r", b=4)
            xv4d = x_d[:, :].rearrange("p (b r) -> p b r", b=4)
            g2b = rpool.tile([128, SLAB * WP], FP8)            # per-group slab
            g2b3 = g2b.rearrange("p (r c) -> p r c", c=WP)

            # slab borders: left/right pad cols everywhere; image-edge pad rows
            nc.vector.memset(g2b3[:, :, 0:1], 0.5)
            nc.vector.memset(g2b3[:, :, WP - 1:WP], 0.5)
            nc.vector.memset(g2b3[0:CI, 0:1, :], 0.5)              # band0 top
            nc.vector.memset(g2b3[96:128, SLAB - 1:SLAB, :], 0.5)  # band3 bottom

            g1_eng = nc.gpsimd if CFG["g1"] == "gpsimd" else nc.vector
            radd_eng = nc.gpsimd if CFG["radd"] == "gpsimd" else nc.vector

            def halo(src_g, src_row, dst_g, dst_row):
                nc.sync.dma_start(
                    out=g2b3[CI * dst_g:CI * (dst_g + 1), dst_row:dst_row + 1, :],
                    in_=g2b3[CI * src_g:CI * (src_g + 1), src_row:src_row + 1, :])

            def stage1(m):
                dn = 2 * m           # local row in down bands (0, 2)
                up = 30 - 2 * m      # local row in up bands (1, 3)
                skip = variant == "nostage1"
                if m % 2 == 0:
                    # band-aware loads: 4 rows per band for macros m, m+1
                    nc.sync.dma_start(
                        out=xv4[:, 0:4:2, 256 * dn:256 * (dn + 4)],
                        in_=xv4d[:, 0:4:2, 256 * dn:256 * (dn + 4)])
                    nc.sync.dma_start(
                        out=xv4[:, 1:4:2, 256 * (up - 2):256 * (up + 2)],
                        in_=xv4d[:, 1:4:2, 256 * (up - 2):256 * (up + 2)])
                if skip:
                    return
                g1t = wpool.tile([C, MACRO], FP8, name="g1t")
                g1_eng.tensor_scalar(
                    out=g1t[:, 0:1024].rearrange("p (b r) -> p b r", b=2),
                    in0=xv4[:, 0:4:2, 256 * dn:256 * (dn + 2)],
                    scalar1=0.0, scalar2=None, op0=OP.is_gt)
                g1_eng.tensor_scalar(
                    out=g1t[:, 1024:2048].rearrange("p (b r) -> p b r", b=2),
                    in0=xv4[:, 1:4:2, 256 * up:256 * (up + 2)],
                    scalar1=0.0, scalar2=None, op0=OP.is_gt)
                c1 = ps1.tile([128, 512], F32, name="c1")
                for g in range(4):
                    nc.tensor.matmul(
                        c1[CI * g:CI * (g + 1), :], w1s,
                        g1t[:, 512 * g:512 * (g + 1)], start=True, stop=True,
                        tile_position=(0, CI * g),
                    )
                # threshold -> padded slabs (down groups 0,1 / up groups 2,3)
                nc.vector.tensor_scalar(
                    out=g2b3[0:64, dn + 1:dn + 3, 1:W + 1],
                    in0=c1[0:64, :], scalar1=t1v[0:64], scalar2=None,
                    op0=OP.is_gt)
                nc.vector.tensor_scalar(
                    out=g2b3[64:128, up + 1:up + 3, 1:W + 1],
                    in0=c1[64:128, :], scalar1=t1v[64:128],
                    scalar2=None, op0=OP.is_gt)
                if m == 0:
                    halo(2, 32, 1, 0)   # h1 row 63 -> band2 top halo
                    halo(1, 1, 2, 33)   # h1 row 64 -> band1 bottom halo
                if m == NMAC - 1:
                    halo(0, 32, 2, 0)   # h1 row 31 -> band1 top halo
                    halo(2, 1, 0, 33)   # h1 row 32 -> band0 bottom halo
                    halo(1, 32, 3, 0)   # h1 row 95 -> band3 top halo
                    halo(3, 1, 1, 33)   # h1 row 96 -> band2 bottom halo

            def stage23(m):
                dn = 2 * m
                up = 30 - 2 * m
                c2 = ps2.tile([128, 512], F32, name="c2")
                if variant == "noconv2":
                    taps = []
                elif variant == "conv2x1":
                    taps = range(3, 6)
                else:
                    taps = range(9)
                ntap = list(taps)
                for t in taps:
                    ky, dx = divmod(t, 3)
                    for g in range(4):
                        lr = dn if g < 2 else up
                        nc.tensor.matmul(
                            c2[CI * g:CI * (g + 1), :],
                            w2s[CI * g:CI * (g + 1), CI * t:CI * (t + 1)],
                            g2b3[CI * g:CI * (g + 1),
                                 lr + ky:lr + ky + 2, dx:dx + W],
                            start=(t == ntap[0]), stop=(t == ntap[-1]),
                            tile_position=(CI * g, CI * g),
                        )
                s3g = wpool.tile([128, 512], FP8, name="s3g")
                if variant != "noconv2":
                    nc.vector.tensor_scalar(out=s3g, in0=c2, scalar1=t2v,
                                            scalar2=None, op0=OP.is_gt)
                for half in range(2):
                    lr = dn if half == 0 else up
                    c3 = ps3.tile([128, 1024], F32, name="c3")
                    for jj in range(2):
                        g = 2 * half + jj
                        nc.tensor.matmul(
                            c3[:, 512 * jj:512 * (jj + 1)],
                            w3s[CI * g:CI * (g + 1), :],
                            s3g[CI * g:CI * (g + 1), :],
                            start=True, stop=True, tile_position=(CI * g, 0),
                        )
                    xap = xv4[:, half:4:2, 256 * lr:256 * (lr + 2)]
                    p3 = epool.tile([128, 1024], F32, name="p3")
                    nc.scalar.activation(p3, c3, AF.Prelu, bias=b3v,
                                         scale=sc3v, alpha=a3)
                    ra = CFG.get("radd", "gpsimd")
                    if variant == "noradd":
                        rt = p3
                    elif ra == "pe":
                        c5 = ps3.tile([128, 1024], F32, name="c3", tag="c3")
                        for b2 in range(2):
                            nc.tensor.matmul(
                                c5[:, 512 * b2:512 * (b2 + 1)], idn,
                                p3[:, 512 * b2:512 * (b2 + 1)],
                                start=True, stop=False)
                            nc.tensor.matmul(
                                c5[:, 512 * b2:512 * (b2 + 1)], idn,
                                xap[:, b2, :], start=False, stop=True)
                        rt = c5
                    else:
                        rt = epool.tile([128, 1024], F32, name="rt")
                        eng = nc.gpsimd if ra == "gpsimd" else nc.vector
                        eng.tensor_tensor(
                            out=rt.rearrange("p (b r) -> p b r", b=2),
                            in0=p3.rearrange("p (b r) -> p b r", b=2),
                            in1=xap, op=OP.add)
                    if variant == "oneprelu":
                        ot = rt
                    else:
                        ot = epool.tile([128, 1024], F32, name="ot")
                        pe2 = CFG.get("prelu2", "scalar")
                        if pe2 == "scalar":
                            nc.scalar.activation(ot, rt, AF.Prelu, alpha=a_out)
                        else:
                            eng = nc.vector if pe2 == "vector" else nc.gpsimd
                            eng.scalar_tensor_tensor(
                                out=ot, in0=rt, scalar=a_out, in1=rt,
                                op0=OP.mult, op1=OP.max)
                    nc.sync.dma_start(
                        out=ov4[:, half:4:2, 256 * lr:256 * (lr + 2)],
                        in_=ot.rearrange("p (b r) -> p b r", b=2))

            LAG = int(CFG.get("lag", 2))

            def whole():
                for m in range(NMAC):
                    stage1(m)
                    if m >= LAG:
                        stage23(m - LAG)
                for m in range(NMAC - LAG, NMAC):
                    stage23(m)

            if repeat == 1:
                whole()
            else:
                with tc.For_i(0, repeat, 1):
                    whole()

    nc.compile()
    return nc


def _host_params(w1, g1, b1, m1, v1, w2, g2, b2, m2, v2, w3, g3, b3, m3, v3):
    def sgn(w):
        return np.where(w <= 0, -1.0, 1.0)

    w1 = np.asarray(w1, np.float64).reshape(CI, C)
    w2 = np.asarray(w2, np.float64).reshape(CI, CI, 3, 3)
    w3 = np.asarray(w3, np.float64).reshape(C, CI)
    s1, s2, s3 = sgn(w1), sgn(w2), sgn(w3)

    def bnfold(g, b, m, v):
        inv = np.asarray(g, np.float64) / np.sqrt(np.asarray(v, np.float64) + EPS)
        beta = np.asarray(b, np.float64) - np.asarray(m, np.float64) * inv
        return inv, beta

    inv1, beta1 = bnfold(g1, b1, m1, v1)
    inv2, beta2 = bnfold(g2, b2, m2, v2)
    inv3, beta3 = bnfold(g3, b3, m3, v3)

    fp8 = ml_dtypes.float8_e4m3
    w1s = (2.0 * s1.T).astype(fp8)                    # [C, CI] lhsT
    # w2s: [128, 9*32]; partitions 32g+c; col block t=(3ky+dx): 2*s2[o,c,ky,dx]
    blk = np.zeros((9, CI, CI), np.float64)
    for ky in range(3):
        for dx in range(3):
            blk[3 * ky + dx] = 2.0 * s2[:, :, ky, dx].T   # [c, o]
    w2st = np.tile(np.concatenate(blk, axis=1), (4, 1)).astype(fp8)  # [128, 288]
    w3st = np.tile(2.0 * s3.T, (4, 1)).astype(fp8)    # [32g+c, o]

    rs1 = s1.sum(axis=1)
    rs2 = s2.sum(axis=(1, 2, 3))
    rs3 = s3.sum(axis=1)

    t1 = np.tile(rs1 - beta1 / inv1, 4).astype(np.float32)
    t2 = np.tile(rs2 - beta2 / inv2, 4).astype(np.float32)
    sc3 = inv3.astype(np.float32)
    b3f = (beta3 - inv3 * rs3).astype(np.float32)
    vecs = np.stack([t1, t2, sc3, b3f], axis=1)       # [C, 4] f32
    return w1s, w2st, w3st, vecs


def _permute_to_bands(img):
    """[C, H, W] -> [C, HW] with bands 0,2,1,3 ... identity: bands are just
    row ranges; no permutation needed (band b = rows 32b..32b+32)."""
    return img.reshape(C, HW)


last_results = None


def kernel(**inputs):
    global last_results
    x = np.ascontiguousarray(np.asarray(inputs["x"], np.float32))
    w1s, w2st, w3st, vecs = _host_params(
        inputs["w1"], inputs["g1"], inputs["b1"], inputs["m1"], inputs["v1"],
        inputs["w2"], inputs["g2"], inputs["b2"], inputs["m2"], inputs["v2"],
        inputs["w3"], inputs["g3"], inputs["b3"], inputs["m3"], inputs["v3"])
    a3 = float(np.asarray(inputs["a3"]))
    a_out = float(np.asarray(inputs["a_out"]))

    key = (a3, a_out)
    if key not in _CACHE:
        _CACHE[key] = _build(a3, a_out)
    nc = _CACHE[key]

    shared = {"w1s": w1s, "w2s": w2st, "w3s": w3st, "vecs": vecs,
              "idn": np.eye(C, dtype=np.float32)}
    in_maps = [dict(x=x[b].reshape(C, HW), **shared) for b in range(B)]
    res = run_bass_kernel_spmd(nc, in_maps, core_ids=list(range(B)))
    last_results = res
    out = np.stack([res.results[b]["out"].reshape(C, H, W) for b in range(B)])
    return out

